# revision 4
# baseline (speedup 1.0000x reference)
"""Channel-attention kernel for Trainium2 (8 NeuronCores, SPMD data-parallel).

out[b] = beta * softmax(rowmax(S) - S, axis=-1) @ x[b] + x[b],  S = x[b] @ x[b].T

Sharding: batch dim B=16 split as 2 batches per core across 8 cores.

v2 changes vs the fp32-I/O baseline:
 - x is loaded as bf16 and out is stored as bf16 (host casts both ways);
   per-core HBM traffic drops 37.75 MB -> 21 MB.  With beta folded into
   the per-row scale, out == bf16(x) when beta == 0, so the rel-err is
   just the bf16 quantization of x (~1e-3).
 - A-transposes are gone: S is symmetric, so the S row-tiles ARE the
   column-tiles.  A^T[d,c] = exp(m[c] - S[d,c]) needs the row stats m[c]
   broadcast along the free dim, done with one tiny PE transpose plus
   four K=1 broadcast matmuls per batch.
 - beta/Z stays a per-partition scalar and is applied in the epilogue
   with a fused scalar_tensor_tensor: out = (f_ps * rzb[c]) + x.
 - x -> fp8 cast for the second matmul runs on GpSimd (otherwise idle).

Both matmul phases run fp8 with perf_mode=DoubleRow (K=256/instruction).
"""

from contextlib import ExitStack

import numpy as np
import ml_dtypes

N_CORES = 8
B, C, N = 16, 512, 4096
BPC = B // N_CORES  # batches per core
P = 128
MT = C // P  # 4 row-blocks of channels
KT = N // P  # 32 partition-tiles of xT
XT_CH = 8  # xT dma/dep chunks (4 k-tiles each)
NQ = N // 512  # 8 n-chunks for the second matmul
KD = C // P  # 4 d-chunks for the second matmul

_CACHE = {}


def _build_bass(reps=1, loop_iters=1, dma_only=False):
    import concourse.bass as bass
    import concourse.bacc as bacc
    import concourse.mybir as mybir
    from concourse import tile, masks

    dt = mybir.dt
    AF = mybir.ActivationFunctionType
    ALU = mybir.AluOpType
    AX = mybir.AxisListType
    DR = mybir.MatmulPerfMode.DoubleRow

    nc = bacc.Bacc(
        "TRN2", target_bir_lowering=False, debug=False, num_devices=N_CORES
    )

    x_dram = nc.dram_tensor("x", [BPC, C, N], dt.bfloat16, kind="ExternalInput")
    # xt is host-pre-shuffled to the exact SBUF tile layout
    # [chunk, partition, k_local, c] so each chunk loads as one fully
    # contiguous 256 KB DMA.
    xt_dram = nc.dram_tensor(
        "xt", [BPC, XT_CH, P, KT // XT_CH, C], dt.float8e4, kind="ExternalInput"
    )
    beta_dram = nc.dram_tensor("beta", [1, 1], dt.float32, kind="ExternalInput")
    out_dram = nc.dram_tensor("out", [BPC, C, N], dt.bfloat16, kind="ExternalOutput")

    with tile.TileContext(nc) as tc, ExitStack() as ctx:
        const_pool = ctx.enter_context(tc.tile_pool(name="const", bufs=1))
        x_pool = ctx.enter_context(tc.tile_pool(name="x", bufs=8))
        xt_pool = ctx.enter_context(tc.tile_pool(name="xt", bufs=2 * XT_CH))
        xb_pool = ctx.enter_context(tc.tile_pool(name="xb", bufs=2))
        o_pool = ctx.enter_context(tc.tile_pool(name="o", bufs=6))
        s_pool = ctx.enter_context(tc.tile_pool(name="s", bufs=8))
        at_pool = ctx.enter_context(tc.tile_pool(name="at", bufs=2))
        scr_pool = ctx.enter_context(tc.tile_pool(name="scr", bufs=2))
        st_pool = ctx.enter_context(tc.tile_pool(name="st", bufs=2))
        spsum = ctx.enter_context(
            tc.tile_pool(name="spsum", bufs=3, space=bass.MemorySpace.PSUM)
        )
        stps = ctx.enter_context(
            tc.tile_pool(name="stps", bufs=1, space=bass.MemorySpace.PSUM)
        )
        fpsum = ctx.enter_context(
            tc.tile_pool(name="fpsum", bufs=3, space=bass.MemorySpace.PSUM)
        )

        # fp32 identity for the stats transpose
        ident = const_pool.tile([P, P], dt.float32)
        masks.make_identity(nc, ident[:])
        # single-partition ones row: stationary operand of the K=1
        # broadcast matmuls
        ones1 = const_pool.tile([1, P], dt.float32)
        nc.gpsimd.memset(ones1[:], 1.0)

        # Broadcast beta scalar to all 128 partitions via ones.T @ beta.
        beta_sb = const_pool.tile([1, 1], dt.float32)
        nc.sync.dma_start(beta_sb[:], beta_dram[:])
        beta_ps = stps.tile([P, 1], dt.float32, tag="t_ps")
        nc.tensor.matmul(beta_ps[:], ones1[:], beta_sb[:], start=True, stop=True)
        beta128 = const_pool.tile([P, 1], dt.float32)
        nc.scalar.copy(beta128[:], beta_ps[:])

        def emit_batch(b):
            # ---- loads ----
            xtt = []
            for ch in range(XT_CH):
                t = xt_pool.tile([P, KT // XT_CH, C], dt.float8e4, tag="xtt")
                nc.sync.dma_start(t[:], xt_dram[b, ch])
                xtt.append(t)
            x_src = x_dram[b].rearrange("(m p) n -> p m n", p=P)
            x_tiles = []
            for m in range(MT):
                xt_t = x_pool.tile([P, N], dt.bfloat16, tag="x")
                nc.sync.dma_start(xt_t[:], x_src[:, m])
                x_tiles.append(xt_t)

            out_dst = out_dram[b].rearrange("(m p) n -> p m n", p=P)
            if dma_only:
                for m in range(MT):
                    nc.sync.dma_start(out_dst[:, m], x_tiles[m][:])
                return

            # ---- cast x -> fp8 on GpSimd for the A @ x moving operand ----
            xb = xb_pool.tile([P, MT, N], dt.float8e4)
            for m in range(MT):
                nc.gpsimd.tensor_copy(xb[:, m, :], x_tiles[m][:])

            # ---- S = x @ x.T  (fp8 DoubleRow, K=256 per instruction) ----
            s_sb = []
            minr = st_pool.tile([P, MT], dt.float32, tag="minr")
            zsum = st_pool.tile([P, MT], dt.float32, tag="z")
            rzb = st_pool.tile([P, MT], dt.float32, tag="rzb")
            for m in range(MT):
                s_ps = spsum.tile([P, 512], dt.float32, tag="s_ps")
                for ch in range(XT_CH):
                    for h in range(2):
                        nc.tensor.matmul(
                            s_ps[:],
                            xtt[ch][:, 2 * h : 2 * h + 2, P * m : P * (m + 1)],
                            xtt[ch][:, 2 * h : 2 * h + 2, :],
                            start=(ch == 0 and h == 0),
                            stop=(ch == XT_CH - 1 and h == 1),
                            perf_mode=DR,
                        )
                # evict S to SBUF (bf16) so the PSUM bank frees early and
                # the transposed re-read runs at DVE 2x
                s_t = s_pool.tile([P, 512], dt.bfloat16, tag="s")
                nc.scalar.copy(s_t[:], s_ps[:])
                s_sb.append(s_t)
                nc.vector.tensor_reduce(
                    minr[:, m : m + 1], s_t[:], axis=AX.X, op=ALU.min
                )
                # Z[c] = sum_d exp(minr[c] - S[c,d]), row layout
                z_scr = scr_pool.tile([P, 512], dt.bfloat16, tag="z_scr")
                nc.scalar.activation(
                    z_scr[:],
                    s_t[:],
                    AF.Exp,
                    bias=minr[:, m : m + 1],
                    scale=-1.0,
                    accum_out=zsum[:, m : m + 1],
                )
            # rzb = beta / Z  (per-partition scale for the epilogue)
            nc.vector.reciprocal(rzb[:], zsum[:])
            nc.vector.tensor_scalar_mul(rzb[:], rzb[:], beta128[:])

            # ---- broadcast minr along the free dim: m_bc[p,c] = minr[c] ----
            # [128,1] -> [1,128] PE transposes (base partition must be 0),
            # then ones.T @ row broadcasts across all 128 partitions.
            mb_ps = stps.tile([P, 512], dt.float32, tag="mb_ps")
            for j in range(MT):
                t1 = stps.tile([1, P], dt.float32, tag="t_ps")
                nc.tensor.transpose(t1[:], minr[:, j : j + 1], ident[:])
                m1 = st_pool.tile([1, P], dt.float32, tag="m1")
                nc.vector.tensor_copy(m1[:], t1[:])
                nc.tensor.matmul(
                    mb_ps[:, P * j : P * (j + 1)],
                    ones1[:],
                    m1[:],
                    start=True,
                    stop=True,
                )
            m_bc = st_pool.tile([P, 512], dt.bfloat16, tag="m_bc")
            nc.scalar.copy(m_bc[:], mb_ps[:])

            # ---- A^T tiles straight from the (symmetric) S tiles ----
            # at[:, j, c] = exp(m_bc[c] - S[d=block j, c])
            at_sb = at_pool.tile([P, KD, 512], dt.float8e4)
            for j in range(KD):
                a_pre = scr_pool.tile([P, 512], dt.bfloat16, tag="a_pre")
                nc.vector.scalar_tensor_tensor(
                    a_pre[:],
                    s_sb[j][:],
                    -1.0,
                    m_bc[:],
                    op0=ALU.mult,
                    op1=ALU.add,
                )
                nc.scalar.activation(at_sb[:, j, :], a_pre[:], AF.Exp)

            # ---- F = A^T.T @ xb, fused epilogue, store ----
            for m in range(MT):
                o_t = o_pool.tile([P, N], dt.bfloat16, tag="o")
                for q in range(NQ):
                    f_ps = fpsum.tile([P, 512], dt.float32, tag="f_ps")
                    for u in range(2):
                        nc.tensor.matmul(
                            f_ps[:],
                            at_sb[:, 2 * u : 2 * u + 2, P * m : P * (m + 1)],
                            xb[:, 2 * u : 2 * u + 2, 512 * q : 512 * (q + 1)],
                            start=(u == 0),
                            stop=(u == 1),
                            perf_mode=DR,
                        )
                    xq = x_tiles[m][:, 512 * q : 512 * (q + 1)]
                    oq = o_t[:, 512 * q : 512 * (q + 1)]
                    if q % 2 == 0:
                        # out = f * (beta/Z) + x in one DVE op
                        nc.vector.scalar_tensor_tensor(
                            oq, f_ps[:], rzb[:, m : m + 1], xq,
                            op0=ALU.mult, op1=ALU.add,
                        )
                    else:
                        # ACT applies the scale, DVE adds x at bf16 2x
                        f_bf = scr_pool.tile([P, 512], dt.bfloat16, tag="f_bf")
                        nc.scalar.activation(
                            f_bf[:], f_ps[:], AF.Copy, scale=rzb[:, m : m + 1]
                        )
                        nc.vector.tensor_add(oq, f_bf[:], xq)
                nc.sync.dma_start(out_dst[:, m], o_t[:])

        def emit_rep():
            for b in range(BPC):
                emit_batch(b)

        if loop_iters > 1:
            with tc.For_i(0, loop_iters, 1):
                for _ in range(reps):
                    emit_rep()
        else:
            for _ in range(reps):
                emit_rep()

    nc.compile()
    return nc


def _get_nc(reps=1, loop_iters=1, dma_only=False):
    key = ("nc", reps, loop_iters, dma_only)
    if key not in _CACHE:
        _CACHE[key] = _build_bass(reps, loop_iters, dma_only)
    return _CACHE[key]


def _make_in_maps(x, beta):
    x = np.ascontiguousarray(x, dtype=np.float32)
    x16 = x.astype(ml_dtypes.bfloat16)
    xt8 = np.ascontiguousarray(
        x.transpose(0, 2, 1), dtype=np.float32
    ).astype(ml_dtypes.float8_e4m3)
    # reorder to the kernel's SBUF tile layout: [b, ch, p, k_local, c]
    # where n = 128 * (4*ch + k_local) + p
    xt8 = np.ascontiguousarray(
        xt8.reshape(B, XT_CH, KT // XT_CH, P, C).transpose(0, 1, 3, 2, 4)
    )
    beta_arr = np.asarray(beta, dtype=np.float32).reshape(1, 1)
    in_maps = []
    for i in range(N_CORES):
        sl = slice(BPC * i, BPC * (i + 1))
        in_maps.append(
            {
                "x": np.ascontiguousarray(x16[sl]),
                "xt": np.ascontiguousarray(xt8[sl]),
                "beta": beta_arr,
            }
        )
    return in_maps


def _run(x, beta, trace=False, **kwargs):
    from concourse.bass_utils import run_bass_kernel_spmd

    nc = _get_nc()
    in_maps = _make_in_maps(x, beta)
    res = run_bass_kernel_spmd(
        nc, in_maps, core_ids=list(range(N_CORES)), trace=trace, **kwargs
    )
    out = np.concatenate([np.asarray(r["out"]) for r in res.results], axis=0)
    return out.astype(np.float32, copy=False), res


def kernel(x, beta):
    out, _ = _run(np.asarray(x), np.asarray(beta))
    return out


# revision 30
# speedup vs baseline: 2.0747x; 2.0747x over previous
"""Channel-attention kernel for Trainium2 (8 NeuronCores, SPMD data-parallel).

out[b] = beta * softmax(rowmax(S) - S, axis=-1) @ x[b] + x[b],  S = x[b] @ x[b].T

Sharding: batch dim B=16 split as 2 batches per core across 8 cores.

v2 changes vs the fp32-I/O baseline:
 - x is loaded as bf16 and out is stored as bf16 (host casts both ways);
   per-core HBM traffic drops 37.75 MB -> 21 MB.  With beta folded into
   the per-row scale, out == bf16(x) when beta == 0, so the rel-err is
   just the bf16 quantization of x (~1e-3).
 - A-transposes are gone: S is symmetric, so the S row-tiles ARE the
   column-tiles.  A^T[d,c] = exp(m[c] - S[d,c]) needs the row stats m[c]
   broadcast along the free dim, done with one tiny PE transpose plus
   four K=1 broadcast matmuls per batch.
 - beta/Z stays a per-partition scalar and is applied in the epilogue
   with a fused scalar_tensor_tensor: out = (f_ps * rzb[c]) + x.
 - x -> fp8 cast for the second matmul runs on GpSimd (otherwise idle).

Both matmul phases run fp8 with perf_mode=DoubleRow (K=256/instruction).
"""

from contextlib import ExitStack

import numpy as np
import ml_dtypes

N_CORES = 8
B, C, N = 16, 512, 4096
BPC = B // N_CORES  # batches per core
P = 128
MT = C // P  # 4 row-blocks of channels
KT = N // P  # 32 partition-tiles of xT
XT_CH = 8  # xT dma/dep chunks (4 k-tiles each)
NQ = N // 512  # 8 n-chunks for the second matmul
KD = C // P  # 4 d-chunks for the second matmul

_CACHE = {}


def _build_bass(reps=1, loop_iters=1, dma_only=False, epi="dve", scast="sv",
                scopy="act", stats="full", arch="row"):
    import concourse.bass as bass
    import concourse.bacc as bacc
    import concourse.mybir as mybir
    from concourse import tile, masks

    dt = mybir.dt
    AF = mybir.ActivationFunctionType
    ALU = mybir.AluOpType
    AX = mybir.AxisListType
    DR = mybir.MatmulPerfMode.DoubleRow

    nc = bacc.Bacc(
        "TRN2", target_bir_lowering=False, debug=False, num_devices=N_CORES
    )

    x_dram = nc.dram_tensor("x", [BPC, C, N], dt.bfloat16, kind="ExternalInput")
    # xt is host-pre-shuffled to the exact SBUF tile layout
    # [chunk, partition, k_local, c] so each chunk loads as one fully
    # contiguous 256 KB DMA.
    xt_dram = nc.dram_tensor(
        "xt", [BPC, XT_CH, P, KT // XT_CH, C], dt.float8e4, kind="ExternalInput"
    )
    beta_dram = nc.dram_tensor("beta", [1, 1], dt.float32, kind="ExternalInput")
    if scast == "hbm":
        # host-precast fp8 copy of x in natural layout [m, p, n]
        xb_dram = nc.dram_tensor(
            "xb", [BPC, MT, P, N], dt.float8e4, kind="ExternalInput"
        )
    out_dram = nc.dram_tensor("out", [BPC, C, N], dt.bfloat16, kind="ExternalOutput")

    with tile.TileContext(nc) as tc, ExitStack() as ctx:
        const_pool = ctx.enter_context(tc.tile_pool(name="const", bufs=1))
        x_pool = ctx.enter_context(tc.tile_pool(name="x", bufs=8))
        xt_pool = ctx.enter_context(tc.tile_pool(name="xt", bufs=2 * XT_CH))
        xb_pool = ctx.enter_context(tc.tile_pool(name="xb", bufs=2))
        o_pool = ctx.enter_context(tc.tile_pool(name="o", bufs=6))
        at_pool = ctx.enter_context(tc.tile_pool(name="at", bufs=2))
        scr_pool = ctx.enter_context(tc.tile_pool(name="scr", bufs=2))
        st_pool = ctx.enter_context(tc.tile_pool(name="st", bufs=2))
        spsum = ctx.enter_context(
            tc.tile_pool(name="spsum", bufs=3, space=bass.MemorySpace.PSUM)
        )
        stps = ctx.enter_context(
            tc.tile_pool(
                name="stps", bufs=(2 if arch == "row" else 1),
                space=bass.MemorySpace.PSUM,
            )
        )
        fpsum = ctx.enter_context(
            tc.tile_pool(name="fpsum", bufs=3, space=bass.MemorySpace.PSUM)
        )
        if arch == "col":
            s_pool = ctx.enter_context(tc.tile_pool(name="s", bufs=8))

        if arch == "row":
            # bf16 identity for the A-block transposes
            ident8 = const_pool.tile([P, P], dt.bfloat16)
            masks.make_identity(nc, ident8[:])
        else:
            # fp32 identity for the stats transpose
            ident = const_pool.tile([P, P], dt.float32)
            masks.make_identity(nc, ident[:])
        # single-partition ones row: stationary operand of the K=1
        # broadcast matmuls
        ones1 = const_pool.tile([1, P], dt.float32)
        nc.gpsimd.memset(ones1[:], 1.0)

        # Broadcast beta scalar to all 128 partitions via ones.T @ beta.
        beta_sb = const_pool.tile([1, 1], dt.float32)
        nc.sync.dma_start(beta_sb[:], beta_dram[:])
        beta_ps = stps.tile([P, 1], dt.float32, tag="t_ps")
        nc.tensor.matmul(beta_ps[:], ones1[:], beta_sb[:], start=True, stop=True)
        beta128 = const_pool.tile([P, 1], dt.float32)
        nc.scalar.copy(beta128[:], beta_ps[:])

        def emit_batch(b):
            # ---- loads ----
            xtt = []
            for ch in range(XT_CH):
                t = xt_pool.tile([P, KT // XT_CH, C], dt.float8e4, tag="xtt")
                nc.sync.dma_start(t[:], xt_dram[b, ch])
                xtt.append(t)
            x_src = x_dram[b].rearrange("(m p) n -> p m n", p=P)
            x_tiles = []
            for m in range(MT):
                xt_t = x_pool.tile([P, N], dt.bfloat16, tag="x")
                nc.sync.dma_start(xt_t[:], x_src[:, m])
                x_tiles.append(xt_t)

            out_dst = out_dram[b].rearrange("(m p) n -> p m n", p=P)
            if dma_only:
                for m in range(MT):
                    nc.sync.dma_start(out_dst[:, m], x_tiles[m][:])
                return

            # ---- x in fp8 natural layout for the A @ x moving operand ----
            xb = xb_pool.tile([P, MT, N], dt.float8e4)
            for m in range(MT):
                if scast == "hbm":
                    nc.sync.dma_start(xb[:, m, :], xb_dram[b, m])
                elif scast == "gpsimd":
                    nc.gpsimd.tensor_copy(xb[:, m, :], x_tiles[m][:])
                elif m % 2 == 0:
                    nc.scalar.copy(xb[:, m, :], x_tiles[m][:])
                else:
                    nc.vector.tensor_copy(xb[:, m, :], x_tiles[m][:])

            minr = st_pool.tile([P, MT], dt.float32, tag="minr")
            zsum = st_pool.tile([P, MT], dt.float32, tag="z")
            rzb = st_pool.tile([P, MT], dt.float32, tag="rzb")
            at_sb = at_pool.tile([P, KD, 512], dt.float8e4)

            if arch == "row":
                # ---- v1-style: row softmax (per-block stats, no barrier),
                # beta/Z folded into A, then PE transposes of A blocks ----
                for m in range(MT):
                    s_ps = spsum.tile([P, 512], dt.float32, tag="s_ps")
                    for ch in range(XT_CH):
                        for h in range(2):
                            nc.tensor.matmul(
                                s_ps[:],
                                xtt[ch][:, 2 * h : 2 * h + 2, P * m : P * (m + 1)],
                                xtt[ch][:, 2 * h : 2 * h + 2, :],
                                start=(ch == 0 and h == 0),
                                stop=(ch == XT_CH - 1 and h == 1),
                                perf_mode=DR,
                            )
                    nc.vector.tensor_reduce(
                        minr[:, m : m + 1], s_ps[:], axis=AX.X, op=ALU.min
                    )
                    a_bf = scr_pool.tile([P, 512], dt.bfloat16, tag="a_bf")
                    nc.scalar.activation(
                        a_bf[:],
                        s_ps[:],
                        AF.Exp,
                        bias=minr[:, m : m + 1],
                        scale=-1.0,
                        accum_out=zsum[:, m : m + 1],
                    )
                    nc.vector.reciprocal(rzb[:, m : m + 1], zsum[:, m : m + 1])
                    nc.vector.tensor_mul(
                        rzb[:, m : m + 1], rzb[:, m : m + 1], beta128[:]
                    )
                    nc.vector.tensor_scalar_mul(
                        a_bf[:], a_bf[:], rzb[:, m : m + 1]
                    )
                    for j in range(KD):
                        t_ps = stps.tile([P, P], dt.bfloat16, tag="t_ps")
                        nc.tensor.transpose(
                            t_ps[:], a_bf[:, P * j : P * (j + 1)], ident8[:]
                        )
                        dst = at_sb[:, j, P * m : P * (m + 1)]
                        if j % 2 == 0:
                            nc.vector.tensor_copy(dst, t_ps[:])
                        else:
                            nc.scalar.copy(dst, t_ps[:])
                # beta/Z already folded into at_sb
                ones_sc = None
            else:
                ones_sc = rzb
                # ---- S = x @ x.T, column-layout A^T via symmetry ----
                s_sb = []
                for m in range(MT):
                    s_ps = spsum.tile([P, 512], dt.float32, tag="s_ps")
                    for ch in range(XT_CH):
                        for h in range(2):
                            nc.tensor.matmul(
                                s_ps[:],
                                xtt[ch][:, 2 * h : 2 * h + 2, P * m : P * (m + 1)],
                                xtt[ch][:, 2 * h : 2 * h + 2, :],
                                start=(ch == 0 and h == 0),
                                stop=(ch == XT_CH - 1 and h == 1),
                                perf_mode=DR,
                            )
                    # evict S to SBUF (bf16) so the PSUM bank frees early
                    # and the transposed re-read runs at DVE 2x
                    s_t = s_pool.tile([P, 512], dt.bfloat16, tag="s")
                    if scopy == "act":
                        nc.scalar.copy(s_t[:], s_ps[:])
                    else:
                        nc.vector.tensor_copy(s_t[:], s_ps[:])
                    s_sb.append(s_t)
                    nc.vector.tensor_reduce(
                        minr[:, m : m + 1], s_t[:], axis=AX.X, op=ALU.min
                    )
                    # Z[c] = sum_d exp(minr[c] - S[c,d]), row layout
                    z_scr = scr_pool.tile([P, 512], dt.bfloat16, tag="z_scr")
                    nc.scalar.activation(
                        z_scr[:],
                        s_t[:],
                        AF.Exp,
                        bias=minr[:, m : m + 1],
                        scale=-1.0,
                        accum_out=zsum[:, m : m + 1],
                    )
                # rzb = beta / Z  (per-partition scale for the epilogue)
                nc.vector.reciprocal(rzb[:], zsum[:])
                nc.vector.tensor_scalar_mul(rzb[:], rzb[:], beta128[:])

            # ---- broadcast minr along the free dim: m_bc[p,c] = minr[c] ----
            if arch == "col":
                m_bc = st_pool.tile([P, 512], dt.bfloat16, tag="m_bc")
                if stats == "memset":
                    # ablation: skip the broadcast chain (timing only)
                    nc.gpsimd.memset(m_bc[:], -300.0)
                else:
                    # One PE transpose [128,4] -> [4,128], then
                    # stream_shuffle moves rows 1-3 to partition 0 so the
                    # K=1 broadcast matmuls (ones.T @ row) can read them.
                    t_ps = stps.tile([MT, P], dt.float32, tag="t_ps")
                    nc.tensor.transpose(t_ps[:], minr[:], ident[:])
                    m_row = st_pool.tile([MT, P], dt.float32, tag="m_row")
                    nc.vector.tensor_copy(m_row[:], t_ps[:])
                    mb_ps = stps.tile([P, 512], dt.float32, tag="mb_ps")
                    for j in range(MT):
                        if j == 0:
                            rj = m_row[0:1, :]
                        else:
                            msh = st_pool.tile([MT, P], dt.float32, tag="msh")
                            nc.vector.stream_shuffle(
                                msh[:], m_row[:], mask=[j] * 32
                            )
                            rj = msh[0:1, :]
                        nc.tensor.matmul(
                            mb_ps[:, P * j : P * (j + 1)],
                            ones1[:],
                            rj,
                            start=True,
                            stop=True,
                        )
                    nc.scalar.copy(m_bc[:], mb_ps[:])

                # ---- A^T tiles straight from the (symmetric) S tiles ----
                # at[:, j, c] = exp(m_bc[c] - S[d=block j, c])
                for j in range(KD):
                    a_pre = scr_pool.tile([P, 512], dt.bfloat16, tag="a_pre")
                    nc.vector.scalar_tensor_tensor(
                        a_pre[:],
                        s_sb[j][:],
                        -1.0,
                        m_bc[:],
                        op0=ALU.mult,
                        op1=ALU.add,
                    )
                    nc.scalar.activation(at_sb[:, j, :], a_pre[:], AF.Exp)

            # ---- F = A^T.T @ xb, fused epilogue, store ----
            for m in range(MT):
                if epi == "none":
                    # ablation: skip the epilogue, store x (timing only)
                    for q in range(NQ):
                        f_ps = fpsum.tile([P, 512], dt.float32, tag="f_ps")
                        for u in range(2):
                            nc.tensor.matmul(
                                f_ps[:],
                                at_sb[:, 2 * u : 2 * u + 2, P * m : P * (m + 1)],
                                xb[:, 2 * u : 2 * u + 2, 512 * q : 512 * (q + 1)],
                                start=(u == 0),
                                stop=(u == 1),
                                perf_mode=DR,
                            )
                    nc.sync.dma_start(out_dst[:, m], x_tiles[m][:])
                    continue
                o_t = o_pool.tile([P, N], dt.bfloat16, tag="o")
                for q in range(NQ):
                    f_ps = fpsum.tile([P, 512], dt.float32, tag="f_ps")
                    for u in range(2):
                        nc.tensor.matmul(
                            f_ps[:],
                            at_sb[:, 2 * u : 2 * u + 2, P * m : P * (m + 1)],
                            xb[:, 2 * u : 2 * u + 2, 512 * q : 512 * (q + 1)],
                            start=(u == 0),
                            stop=(u == 1),
                            perf_mode=DR,
                        )
                    xq = x_tiles[m][:, 512 * q : 512 * (q + 1)]
                    oq = o_t[:, 512 * q : 512 * (q + 1)]
                    if arch == "row":
                        if epi == "dve" or q % 2 == 0:
                            # beta/Z already folded into A: plain add
                            nc.vector.tensor_add(oq, f_ps[:], xq)
                        else:
                            f_bf = scr_pool.tile(
                                [P, 512], dt.bfloat16, tag="f_bf"
                            )
                            nc.scalar.copy(f_bf[:], f_ps[:])
                            nc.vector.tensor_add(oq, f_bf[:], xq)
                    elif epi == "dve" or q % 2 == 0:
                        # out = f * (beta/Z) + x in one DVE op
                        nc.vector.scalar_tensor_tensor(
                            oq, f_ps[:], rzb[:, m : m + 1], xq,
                            op0=ALU.mult, op1=ALU.add,
                        )
                    else:
                        # ACT applies the scale, DVE adds x at bf16 2x
                        f_bf = scr_pool.tile([P, 512], dt.bfloat16, tag="f_bf")
                        nc.scalar.activation(
                            f_bf[:], f_ps[:], AF.Copy, scale=rzb[:, m : m + 1]
                        )
                        nc.vector.tensor_add(oq, f_bf[:], xq)
                nc.sync.dma_start(out_dst[:, m], o_t[:])

        def emit_rep():
            for b in range(BPC):
                emit_batch(b)

        if loop_iters > 1:
            with tc.For_i(0, loop_iters, 1):
                for _ in range(reps):
                    emit_rep()
        else:
            for _ in range(reps):
                emit_rep()

    nc.compile()
    return nc


def _get_nc(reps=1, loop_iters=1, **kw):
    key = ("nc", reps, loop_iters, tuple(sorted(kw.items())))
    if key not in _CACHE:
        _CACHE[key] = _build_bass(reps, loop_iters, **kw)
    return _CACHE[key]


def _make_in_maps(x, beta):
    x = np.ascontiguousarray(x, dtype=np.float32)
    x16 = x.astype(ml_dtypes.bfloat16)
    xt8 = np.ascontiguousarray(
        x.transpose(0, 2, 1), dtype=np.float32
    ).astype(ml_dtypes.float8_e4m3)
    # reorder to the kernel's SBUF tile layout: [b, ch, p, k_local, c]
    # where n = 128 * (4*ch + k_local) + p
    xt8 = np.ascontiguousarray(
        xt8.reshape(B, XT_CH, KT // XT_CH, P, C).transpose(0, 1, 3, 2, 4)
    )
    beta_arr = np.asarray(beta, dtype=np.float32).reshape(1, 1)
    xb8 = x.astype(ml_dtypes.float8_e4m3).reshape(B, MT, P, N)
    in_maps = []
    for i in range(N_CORES):
        sl = slice(BPC * i, BPC * (i + 1))
        in_maps.append(
            {
                "x": np.ascontiguousarray(x16[sl]),
                "xt": np.ascontiguousarray(xt8[sl]),
                "xb": np.ascontiguousarray(xb8[sl]),
                "beta": beta_arr,
            }
        )
    return in_maps


def _run(x, beta, trace=False, **kwargs):
    from concourse.bass_utils import run_bass_kernel_spmd

    nc = _get_nc()
    in_maps = _make_in_maps(x, beta)
    res = run_bass_kernel_spmd(
        nc, in_maps, core_ids=list(range(N_CORES)), trace=trace, **kwargs
    )
    out = np.concatenate([np.asarray(r["out"]) for r in res.results], axis=0)
    return out.astype(np.float32, copy=False), res


def kernel(x, beta):
    out, _ = _run(np.asarray(x), np.asarray(beta))
    return out
